# revision 1
# baseline (speedup 1.0000x reference)
"""Deductron (sigmoid-gated affine linear recurrence) — Trainium2 Bass kernel.

Problem: T=524288, INPUT_LEN=64, N_MEMORY=64, OUTPUT_LEN=32.
  h = sigmoid(x @ W1 + B1); l, r = split(h); a = (l*r)[:-1]; b = (1-l)[:-1]
  u_t = a_{t-1} u_{t-1} + b_{t-1}, u_0 = 0;  out = z @ W2 + B2

Strategy (8 NeuronCores, sequence-parallel, no collectives):
  - a_t = sigmoid*sigmoid < ~0.6, so state influence decays geometrically;
    a warm-up halo of W=512 steps makes chunks independent to f32 precision
    (decay < 1e-45). Core 0's halo coefficients are zeroed via a mask input
    so its first sub-block starts at exactly u=0.
  - Each core handles C=65536 rows as two packed sub-blocks of NP=32768
    (128 partitions = 2 sub-blocks x 64 channels); the host pre-transposes
    x into this packed layout (xt [128, W+NP]).
  - Gating: block-diagonal W1-half matmuls (K=128 covers both sub-blocks),
    fp16 operands -> single-pass matmuls (fp32 would emit LO/HI pairs).
  - ScalarE: l = sigmoid(zl+B1), r = sigmoid(zr+B1r), and
    b = sigmoid(-zl-B1) (= 1-l exactly) -- three activations per tile.
  - VectorE: a = l*r (fp16 2x mode) + the recurrence via tensor_tensor_scan
    (fp32 internal state, fp16 in/out, HW rate ~2.1 cy/col). Shifted-output
    convention: scan col k = z[row+k+1], so no carry copies are needed.
  - Output: z streams to DRAM as fp16 (same bytes as the f32 out would be);
    the host finishes the small z @ W2 + B2 projection during gather
    (host_w2=True). A device-side W2 path is kept behind host_w2=False.
  - Steady state is jointly limited by ScalarE (3 sigmoids ~5.7us/iter) and
    VectorE (scan 4.5 + mul 1.2us/iter); GpSimd offload loses to DVE<->Q7
    SBUF port contention.
"""

import os
import sys
from dataclasses import dataclass

for _p in ("/opt/trn_rl_repo",):
    if _p not in sys.path and os.path.isdir(_p):
        sys.path.insert(0, _p)

import numpy as np

import concourse.bacc as bacc
import concourse.mybir as mybir
import concourse.tile as tile
from concourse.bass_utils import run_bass_kernel_spmd

F32 = mybir.dt.float32
F16 = mybir.dt.float16
AF = mybir.ActivationFunctionType
OP = mybir.AluOpType


@dataclass
class Cfg:
    C: int  # rows per core
    W: int  # warm-up halo steps
    NT: int  # time-steps per iteration tile (per sub-block)
    NCH: int = 64
    NOUT: int = 32
    fp16: bool = True  # 16-bit gating/coeff/scan/W2 path
    amul_pool: bool = False  # split a = l*r across GpSimd/VectorE
    host_w2: bool = True  # device emits z (fp16); host does z @ W2 + B2
    b_sigmoid: bool = True  # b = sigmoid(-zl) on ScalarE (else 1-l on DVE)
    a_gpsimd: bool = False  # a = l*r fully on GpSimd (else VectorE)

    @property
    def NP(self):
        return self.C // 2

    @property
    def NITER(self):
        assert self.NP % self.NT == 0
        return self.NP // self.NT

    @property
    def NBJ(self):
        assert self.NT % 128 == 0
        return self.NT // 128


FULL = Cfg(C=65536, W=128, NT=2048)
N_CORES = 8
T = 524288


def build_deductron(tc, io, cfg: Cfg):
    """Emit the kernel. io: dict of DRAM APs: xt, w1bdl, w1bdr, b1l, b1r,
    w2bd, b2rep, mask, out.

    Shifted-output convention: scan-out col k of iteration i = z[row0+k+1]
    where row0 = sub-block start + i*NT. Each core writes local out rows
    [1, C]; the host stitches (global row 0 = B2, core row 0 unused).
    """
    nc = tc.nc
    NT, W, NBJ = cfg.NT, cfg.W, cfg.NBJ
    DT = F16 if cfg.fp16 else F32
    NH = NT // 2

    xt_d = io["xt"]
    out_d = io["out"]

    with (
        tc.tile_pool(name="consts", bufs=1) as cpool,
        tc.tile_pool(name="xt", bufs=4) as xpool,
        tc.tile_pool(name="lr", bufs=2) as lrpool,
        tc.tile_pool(name="ab", bufs=2) as abpool,
        tc.tile_pool(name="z", bufs=2) as zpool,
        tc.tile_pool(name="osb", bufs=3) as opool,
        tc.tile_pool(name="pzl", bufs=1, space="PSUM") as pzl,
        tc.tile_pool(name="pzr", bufs=1, space="PSUM") as pzr,
        tc.tile_pool(name="pout", bufs=1, space="PSUM") as pout,
    ):
        c16 = cpool.tile([128, 256], DT, tag="c16")  # [w1bdl | w1bdr]
        c32 = cpool.tile([128, 4], F32, tag="c32")  # [b1l|b1ln|b1r|mask]
        nc.sync.dma_start(c16[:], io["c16"])
        nc.sync.dma_start(c32[:], io["c32"])
        w1bdl, w1bdr = c16[:, 0:128], c16[:, 128:256]
        b1l, b1ln, b1r = c32[:, 0:1], c32[:, 1:2], c32[:, 2:3]
        mask = c32[:, 3:4]
        if not cfg.host_w2:
            w2bd = cpool.tile([128, 64], DT, tag="w2bd")
            b2rep = cpool.tile([128, NH], F32, tag="b2rep")
            nc.sync.dma_start(w2bd[:], io["w2bd"])
            nc.sync.dma_start(b2rep[:], io["b2rep"])

        def gate_L(xt_t, n, apply_mask):
            # one [128, n<=NT] psum tile, single sigmoid inst
            zl_t = pzl.tile([128, NT], F32, tag="zl")
            l_t = lrpool.tile([128, NT], DT, tag="l")
            for q0 in range(0, n, 512):
                q1 = min(q0 + 512, n)
                nc.tensor.matmul(
                    zl_t[:, q0:q1], w1bdl, xt_t[:, q0:q1], start=True, stop=True
                )
            nc.scalar.activation(
                l_t[:, 0:n], zl_t[:, 0:n], AF.Sigmoid, bias=b1l
            )
            b_t = abpool.tile([128, NT], DT, tag="b")
            if cfg.b_sigmoid:
                # b = 1 - sigmoid(zl+B1) = sigmoid(-zl-B1); keeps b off VectorE
                nc.scalar.activation(
                    b_t[:, 0:n], zl_t[:, 0:n], AF.Sigmoid, bias=b1ln,
                    scale=-1.0,
                )
            else:
                # b = 1 - l on DVE: fp16 tensor_scalar runs in 4x mode
                # (~0.26 cy/col), far cheaper than a third ScalarE sigmoid
                nc.vector.tensor_scalar(
                    b_t[:, 0:n], l_t[:, 0:n], -1.0, 1.0, op0=OP.mult, op1=OP.add
                )
            if apply_mask:  # zero core-0's warm-up cols (first W only)
                nc.vector.tensor_scalar(
                    b_t[:, 0:W], b_t[:, 0:W], mask, None, op0=OP.mult
                )
            return l_t, b_t

        def gate_R(xt_t, n):
            r_t = lrpool.tile([128, NT], DT, tag="r")
            step = NT if cfg.host_w2 else NH
            for h0 in range(0, n, step):
                h1 = min(h0 + step, n)
                zr_t = pzr.tile([128, step], F32, tag="zr")
                for q0 in range(h0, h1, 512):
                    q1 = min(q0 + 512, h1)
                    nc.tensor.matmul(
                        zr_t[:, q0 - h0 : q1 - h0],
                        w1bdr,
                        xt_t[:, q0:q1],
                        start=True,
                        stop=True,
                    )
                nc.scalar.activation(
                    r_t[:, h0:h1], zr_t[:, 0 : h1 - h0], AF.Sigmoid, bias=b1r
                )
            return r_t

        def coeff_a(l_t, r_t, n, apply_mask):
            # split across GpSimd and VectorE: Q7 contends with DVE's
            # TT/TS port usage but not with the scan, so it absorbs most
            # of the multiply while DVE runs the recurrence
            a_t = abpool.tile([128, NT], DT, tag="a")
            POOL_COLS = (n // 512) * 384
            if cfg.a_gpsimd:
                nc.gpsimd.tensor_mul(a_t[:, 0:n], l_t[:, 0:n], r_t[:, 0:n])
            elif cfg.amul_pool and POOL_COLS:
                nc.gpsimd.tensor_mul(
                    a_t[:, 0:POOL_COLS], l_t[:, 0:POOL_COLS], r_t[:, 0:POOL_COLS]
                )
                nc.vector.tensor_mul(
                    a_t[:, POOL_COLS:n], l_t[:, POOL_COLS:n], r_t[:, POOL_COLS:n]
                )
            else:
                nc.vector.tensor_mul(a_t[:, 0:n], l_t[:, 0:n], r_t[:, 0:n])
            if apply_mask:  # zero core-0's warm-up cols (first W only)
                nc.vector.tensor_scalar(
                    a_t[:, 0:W], a_t[:, 0:W], mask, None, op0=OP.mult
                )
            return a_t

        # ---------------- halo ----------------
        xt_h = xpool.tile([128, NT], DT, tag="xt")
        nc.sync.dma_start(xt_h[:, 0:W], xt_d[:, 0:W])
        l_h, b_h = gate_L(xt_h, W, apply_mask=True)
        r_h = gate_R(xt_h, W)
        a_h = coeff_a(l_h, r_h, W, apply_mask=True)
        z_prev = zpool.tile([128, NT], DT, tag="z")
        nc.vector.tensor_tensor_scan(
            z_prev[:, 0:W], a_h[:, 0:W], b_h[:, 0:W], 0.0, op0=OP.mult, op1=OP.add
        )
        prev_last = W  # z_prev[:, prev_last-1] holds the carry

        # ---------------- main loop ----------------
        # short first iterations cut the pipeline-fill latency (the DMA ->
        # matmul -> sigmoid -> mul -> scan chain is ~12us deep at full size);
        # a split last iteration shortens the serial scan+DMA drain
        ramp = [256, 256, 512, 1024]
        assert sum(ramp) == cfg.NT
        sizes = ramp + [cfg.NT] * (cfg.NITER - 2) + [cfg.NT // 2, cfg.NT // 2]
        assert sum(sizes) == cfg.NP
        c0 = W
        for i, n in enumerate(sizes):
            xt_t = xpool.tile([128, NT], DT, tag="xt")
            nc.sync.dma_start(xt_t[:, 0:n], xt_d[:, c0 : c0 + n])
            l_t, b_t = gate_L(xt_t, n, apply_mask=False)
            r_t = gate_R(xt_t, n)
            a_t = coeff_a(l_t, r_t, n, apply_mask=False)

            z_t = zpool.tile([128, NT], DT, tag="z")
            nc.vector.tensor_tensor_scan(
                z_t[:, 0:n],
                a_t[:, 0:n],
                b_t[:, 0:n],
                z_prev[:, prev_last - 1 : prev_last],
                op0=OP.mult,
                op1=OP.add,
            )

            if cfg.host_w2:
                # stream z straight out; host applies z @ W2 + B2 in gather
                nc.sync.dma_start(
                    out_d[:, c0 - W : c0 - W + n], z_t[:, 0:n]
                )
            else:
                r0 = c0 - W  # local row base (pre-shift)
                nbj = n // 128
                out_ps = pout.tile([128, NH], F32, tag="outp")
                for j in range(nbj):
                    nc.tensor.matmul(
                        out_ps[:, j * 64 : (j + 1) * 64],
                        z_t[:, j * 128 : (j + 1) * 128],
                        w2bd[:],
                        start=True,
                        stop=True,
                    )
                out_sb = opool.tile([128, NH], F32, tag="osb")
                nc.vector.tensor_add(
                    out_sb[:, 0 : nbj * 64], out_ps[:, 0 : nbj * 64],
                    b2rep[:, 0 : nbj * 64],
                )
                osb3 = out_sb[:, 0 : nbj * 64].rearrange("p (j c) -> p j c", c=64)
                outA = out_d[r0 + 1 : r0 + n + 1, :].rearrange(
                    "(j p) c -> p j c", p=128
                )
                outB = out_d[cfg.NP + r0 + 1 : cfg.NP + r0 + n + 1, :].rearrange(
                    "(j p) c -> p j c", p=128
                )
                nc.sync.dma_start(outA, osb3[:, :, 0:32])
                nc.sync.dma_start(outB, osb3[:, :, 32:64])

            z_prev, prev_last = z_t, n
            c0 += n


def prep_inputs(x, W1, B1, W2, B2, cfg: Cfg, n_cores: int):
    """Host-side prep: per-core packed transposed x + block-diag weights."""
    x = np.asarray(x, np.float32)
    W1 = np.asarray(W1, np.float32)
    B1 = np.asarray(B1, np.float32)
    W2 = np.asarray(W2, np.float32)
    B2 = np.asarray(B2, np.float32)
    NCH, NP, W, C = cfg.NCH, cfg.NP, cfg.W, cfg.C
    ndt = np.float16 if cfg.fp16 else np.float32

    W1L, W1R = W1[:, :NCH], W1[:, NCH:]
    w1bdl = np.zeros((128, 128), ndt)
    w1bdl[:64, :64] = W1L
    w1bdl[64:, 64:] = W1L
    w1bdr = np.zeros((128, 128), ndt)
    w1bdr[:64, :64] = W1R
    w1bdr[64:, 64:] = W1R
    w2bd = np.zeros((128, 64), ndt)
    w2bd[:64, :32] = W2
    w2bd[64:, 32:] = W2
    b1l = np.tile(B1[0, :NCH], 2).reshape(128, 1).astype(np.float32)
    b1ln = -b1l
    b1r = np.tile(B1[0, NCH:], 2).reshape(128, 1).astype(np.float32)
    b2rep = np.tile(np.concatenate([B2[0], B2[0]]), cfg.NBJ).reshape(1, -1)
    b2rep = np.broadcast_to(b2rep, (128, cfg.NBJ * 64)).astype(np.float32).copy()

    c16 = np.concatenate([w1bdl, w1bdr], axis=1)  # [128, 256]
    in_maps = []
    for c in range(n_cores):
        sA = c * C
        sB = sA + NP
        if c == 0:
            xa = np.concatenate([np.zeros((W, NCH), np.float32), x[0 : sA + NP]], 0)
            m = np.concatenate(
                [np.zeros(64, np.float32), np.ones(64, np.float32)]
            ).reshape(128, 1)
        else:
            xa = x[sA - W : sA + NP]
            m = np.ones((128, 1), np.float32)
        xb = x[sB - W : sB + NP]
        xt = np.ascontiguousarray(np.concatenate([xa.T, xb.T], 0).astype(ndt))
        c32 = np.concatenate([b1l, b1ln, b1r, m], axis=1)  # [128, 4]
        in_maps.append(
            {
                "xt": xt,
                "c16": c16,
                "c32": np.ascontiguousarray(c32),
                "w2bd": w2bd,
                "b2rep": b2rep,
            }
        )
    return in_maps


def declare_io(nc, cfg: Cfg):
    DT = mybir.dt.float16 if cfg.fp16 else F32
    io = {
        "xt": nc.dram_tensor("xt", [128, cfg.W + cfg.NP], DT, kind="ExternalInput"),
        "c16": nc.dram_tensor("c16", [128, 256], DT, kind="ExternalInput"),
        "c32": nc.dram_tensor("c32", [128, 4], F32, kind="ExternalInput"),
        "w2bd": nc.dram_tensor("w2bd", [128, 64], DT, kind="ExternalInput"),
        "b2rep": nc.dram_tensor(
            "b2rep", [128, cfg.NBJ * 64], F32, kind="ExternalInput"
        ),
        "out": (
            nc.dram_tensor("out", [128, cfg.NP], mybir.dt.float16, kind="ExternalOutput")
            if cfg.host_w2
            else nc.dram_tensor("out", [cfg.C + 1, cfg.NOUT], F32, kind="ExternalOutput")
        ),
    }
    return {k: v.ap() for k, v in io.items()}


_NC = None
LAST_RESULTS = None


def _get_nc():
    global _NC
    if _NC is None:
        nc = bacc.Bacc(
            "TRN2", target_bir_lowering=False, debug=False, num_devices=N_CORES
        )
        io = declare_io(nc, FULL)
        with tile.TileContext(nc) as tc:
            build_deductron(tc, io, FULL)
        nc.compile()
        _NC = nc
    return _NC


def kernel(inputs, W1, B1, W2, B2):
    global LAST_RESULTS
    nc = _get_nc()
    in_maps = prep_inputs(inputs, W1, B1, W2, B2, FULL, N_CORES)
    trace = bool(int(os.environ.get("KERNEL_TRACE", "0")))
    res = run_bass_kernel_spmd(
        nc, in_maps, core_ids=list(range(N_CORES)), trace=trace
    )
    LAST_RESULTS = res
    if FULL.host_w2:
        # device emitted z in packed-transposed fp16; finish z @ W2 + B2 here
        W2f = np.asarray(W2, np.float32)
        B2f = np.asarray(B2, np.float32).reshape(-1)
        z = np.empty((T + 1, 64), np.float32)
        z[0] = 0.0
        NP = FULL.NP
        for c in range(N_CORES):
            zc = res.results[c]["out"]  # [128, NP] fp16; col k -> z[start+k+1]
            sA = c * FULL.C
            z[sA + 1 : sA + NP + 1] = zc[0:64].T
            z[sA + NP + 1 : sA + 2 * NP + 1] = zc[64:128].T
        return (z[:T] @ W2f + B2f).astype(np.float32)
    out = np.empty((T, FULL.NOUT), np.float32)
    out[0] = np.asarray(B2, np.float32).reshape(-1)
    for c in range(N_CORES):
        lo = c * FULL.C + 1
        hi = min(lo + FULL.C, T)
        out[lo:hi] = res.results[c]["out"][1 : 1 + hi - lo]
    return out



# revision 3
# speedup vs baseline: 1.4598x; 1.4598x over previous
"""Deductron (sigmoid-gated affine linear recurrence) — Trainium2 Bass kernel.

Problem: T=524288, INPUT_LEN=64, N_MEMORY=64, OUTPUT_LEN=32.
  h = sigmoid(x @ W1 + B1); l, r = split(h); a = (l*r)[:-1]; b = (1-l)[:-1]
  u_t = a_{t-1} u_{t-1} + b_{t-1}, u_0 = 0;  out = z @ W2 + B2

Strategy (8 NeuronCores, sequence-parallel, no collectives):
  - a_t = sigmoid*sigmoid decays geometrically; a warm-up halo of W=128
    steps makes chunks independent to f32 precision. Core 0's chunk-0 halo
    is forced to (a=0, b=0) via l:=1, r:=0 with a per-core mask input.
  - Each core handles C=65536 rows as FOUR chunks of NQ=16384: 2 chunks in
    the partition dim (128 partitions = 2 blocks x 64 channels) and 2 chunks
    INTERLEAVED along columns (even cols = stream A, odd = stream B).
  - Gating: block-diagonal W1-half matmuls (fp16, K=128), two sigmoids on
    ScalarE (l and r) — the bottleneck engine at ~1 elem/cy/lane.
  - The recurrence runs in ONE custom DVE instruction per tile
    (DEDUCTRON_ISCAN2_ANT, see _register_fused_scan): a hand-written uOp
    program computing u <- (l*r)*u + (1-l) over the two interleaved streams
    at 1 elem/cycle — the stock tensor_tensor_scan needs a bubble uOp
    (~2.1 cy/elem) because its feedback distance is 1 element; interleaving
    two streams makes the NEXT_ALU_OUT_A feedback distance (2 elements)
    exactly right. This also fuses a=l*r and b=1-l, freeing DVE and ScalarE.
  - Seed convention: scan cols 0,1 = per-stream carries, passed via the l
    stream (copied from the previous tile's last two states).
  - Output: z streams to DRAM as fp16; the host finishes z @ W2 + B2 and
    the de-interleave during gather.
"""

import os
import sys

for _p in ("/opt/trn_rl_repo",):
    if _p not in sys.path and os.path.isdir(_p):
        sys.path.insert(0, _p)

import numpy as np

import concourse.bacc as bacc
import concourse.mybir as mybir
import concourse.tile as tile
from concourse.bass_utils import run_bass_kernel_spmd

F32 = mybir.dt.float32
F16 = mybir.dt.float16
AF = mybir.ActivationFunctionType
OP = mybir.AluOpType

# ---------------------------------------------------------------------------
# Custom DVE op: fused 2-interleaved affine scan (hand-written uOp program).
#
# Semantics over one [P, N] pair of streams l (Src0), r (Src1):
#     out[:, 0] = l[:, 0]; out[:, 1] = l[:, 1]          # seeds (carries)
#     out[:, k] = (l*r)[:, k] * out[:, k-2] + (1-l)[:, k]   for k >= 2
# fp32 internal state, 1 element/cycle.
#
# Datapath (v3): state = blk3's a_flop, read by blk2 via NEXT_ALU_OUT_A —
# with elements issued every cycle, the value read is 2 elements stale,
# which is exactly the same-stream predecessor under 2-way interleave.
#     lanes: inp0=SRC_0(l), inp1=SRC_1(r)→ch0, inp2=ONE→ch1, inp3=ZERO→ch2
#     blk0: p = l * r                       capture l→ch3
#     blk1: t = 1 - l    (SUB ch1, ch3)     capture p→ch4
#     blk2: m = p * state (MUL ch4, NEXT_A) capture t→ch5
#     blk3: u = m + t  → out flop + a_flop
#     blk4-7: bypass → WR0_LO
# Seed uOp (repeat 2, consumes + writes): blk1 = BYPASS(ch3)=l,
# blk2 = BYPASS(ch2)=0 — never reads the stale a_flop.
# ---------------------------------------------------------------------------


def _register_fused_scan():
    from concourse.dve_ops import (
        OPS,
        CUSTOM_DVE_SPECS,
        _SUB_OPCODE_FOR_NAME,
        DveOp,
        get_dve_sub_opcode,
    )
    from concourse.dve_spec import Spec, Src0, Src1
    from concourse.dve_uop import (
        ENABLE,
        AluInp,
        AluOp,
        DelayInp,
        DveOpSpec,
        InpSel,
        OutPath,
        OutSel,
        Trigger,
        UopConfig,
        UopDpConfig,
    )

    name = "DEDUCTRON_ISCAN2_ANT"
    if name in _SUB_OPCODE_FOR_NAME:  # already registered (re-import)
        return next(op for op in OPS if op.name == name)

    def _dp():
        dp = [UopDpConfig() for _ in range(8)]
        dp[0].enable_alu(AluOp.MULTIPLY, AluInp.PREV_ALU_OUT, AluInp.PREV_DELAY_0)
        dp[0].pass_through_delay(1, 2)
        dp[0].enable_delay_from_src(DelayInp.PREV_ALU_OUT, 3)
        dp[1].enable_alu(AluOp.SUBTRACT, AluInp.PREV_DELAY_1, AluInp.PREV_DELAY_3)
        dp[1].pass_through_delay(2)
        dp[1].enable_delay_from_src(DelayInp.PREV_ALU_OUT, 4)
        dp[2].enable_alu(AluOp.MULTIPLY, AluInp.PREV_DELAY_4, AluInp.NEXT_ALU_OUT_A)
        dp[2].enable_delay_from_src(DelayInp.PREV_ALU_OUT, 5)
        dp[3].enable_alu(AluOp.ADD, AluInp.PREV_ALU_OUT, AluInp.PREV_DELAY_5)
        dp[3].alu_out_a_enable = ENABLE
        for i in range(4, 8):
            dp[i].pass_through_alu()
        return dp

    def _uop(dp):
        u = UopConfig(datapath_config=dp)
        u.enable_input(InpSel.SRC_0, 0)
        u.enable_input(InpSel.SRC_1, 1)
        u.enable_input(InpSel.ONE_F32, 2)
        u.enable_input(InpSel.ZERO, 3)
        u.enable_output(OutSel.ALU_OUT, OutPath.WR0_LO)
        u.require_inp0 = 1
        u.require_inp1 = 1
        return u

    def _uops():
        seed = _uop(_dp())
        seed.datapath_config[1].enable_alu(
            AluOp.BYPASS, AluInp.PREV_DELAY_3, AluInp.PREV_DELAY_3
        )
        seed.datapath_config[2].enable_alu(
            AluOp.BYPASS, AluInp.PREV_DELAY_2, AluInp.PREV_DELAY_2
        )
        seed.repeat_count = 2
        seed.trigger = (Trigger.COUNT, Trigger.NONE, Trigger.NONE)
        seed.next_uop = (1, 0, 0)
        steady = _uop(_dp())
        steady.trigger = (Trigger.SRC_TENSOR_DONE, Trigger.NONE, Trigger.NONE)
        steady.next_uop = (0, 0, 0)
        return [seed, steady]

    def _reference(in0, in1, c0, c1, c2):
        l = np.asarray(in0, np.float32)
        r = np.asarray(in1, np.float32)
        P = l.shape[0]
        l2, r2 = l.reshape(P, -1), r.reshape(P, -1)
        out = np.empty_like(l2)
        out[:, 0:2] = l2[:, 0:2]
        for k in range(2, l2.shape[1]):
            out[:, k] = l2[:, k] * r2[:, k] * out[:, k - 2] + (1.0 - l2[:, k])
        return out.reshape(l.shape)

    class _HandDveOp(DveOp):
        def compile(self, ver):
            assert ver == "v3", f"{self.name}: hand-written program targets v3/TRN2"
            s = DveOpSpec(
                name=self.name,
                opcode=get_dve_sub_opcode(self.name),
                uops=_uops(),
                rd1_en=True,
            )
            s.validate(ver)
            return s

    op = _HandDveOp(
        name, Spec(body=Src0 * Src1, reference=_reference), subdim=False, uops_sha={}
    )
    row = max(_SUB_OPCODE_FOR_NAME.values()) + 1
    assert row < 0x20
    _SUB_OPCODE_FOR_NAME[name] = row
    OPS.append(op)
    CUSTOM_DVE_SPECS[name] = op.spec
    return op


FUSED_SCAN = _register_fused_scan()

# ---------------------------------------------------------------------------
# Kernel
# ---------------------------------------------------------------------------

N_CORES = 8
T = 524288
NCH = 64
C = T // N_CORES  # 65536 rows per core
NQ = C // 4  # 16384 rows per chunk (4 chunks per core)
W = 128  # warm-up halo steps per chunk
HB = 2 * W  # interleaved halo columns
NCOL = 2 * (W + NQ)  # xt columns per core (33024)
NREAL = 2 * NQ  # real (output) columns per core (32768)
NT = 2048  # main-loop tile columns


def build_deductron(tc, io):
    """io: dict of DRAM APs: xt [128, NCOL] f16, c16 [128, 258] f16
    (w1bdl | w1bdr | zeros2), c32 [128, 4] f32 (b1l | b1r | m | mm),
    out [128, NREAL] f16 (interleaved z, shifted by one row)."""
    nc = tc.nc

    with (
        tc.tile_pool(name="consts", bufs=1) as cpool,
        tc.tile_pool(name="xt", bufs=4) as xpool,
        tc.tile_pool(name="lr", bufs=2) as lrpool,
        tc.tile_pool(name="z", bufs=2) as zpool,
        tc.tile_pool(name="pzl", bufs=1, space="PSUM") as pzl,
        tc.tile_pool(name="pzr", bufs=1, space="PSUM") as pzr,
    ):
        c16 = cpool.tile([128, 258], F16, tag="c16")
        c32 = cpool.tile([128, 4], F32, tag="c32")
        nc.sync.dma_start(c16[:], io["c16"])
        nc.sync.dma_start(c32[:], io["c32"])
        w1bdl, w1bdr, zeros2 = c16[:, 0:128], c16[:, 128:256], c16[:, 256:258]
        b1l, b1r = c32[:, 0:1], c32[:, 1:2]
        m, mm = c32[:, 2:3], c32[:, 3:4]

        def gates(xt_t, n):
            """matmul + sigmoid for both halves; returns l_t, r_t with the
            gate values at cols [2, 2+n) (cols 0,1 reserved for carries)."""
            l_t = lrpool.tile([128, NT + 2], F16, tag="l")
            r_t = lrpool.tile([128, NT + 2], F16, tag="r")
            zl_t = pzl.tile([128, NT], F32, tag="zl")
            zr_t = pzr.tile([128, NT], F32, tag="zr")
            for q0 in range(0, n, 512):
                q1 = min(q0 + 512, n)
                nc.tensor.matmul(
                    zl_t[:, q0:q1], w1bdl, xt_t[:, q0:q1], start=True, stop=True
                )
                nc.tensor.matmul(
                    zr_t[:, q0:q1], w1bdr, xt_t[:, q0:q1], start=True, stop=True
                )
            nc.scalar.activation(
                l_t[:, 2 : 2 + n], zl_t[:, 0:n], AF.Sigmoid, bias=b1l
            )
            nc.scalar.activation(
                r_t[:, 2 : 2 + n], zr_t[:, 0:n], AF.Sigmoid, bias=b1r
            )
            return l_t, r_t

        # ---------------- halo block ----------------
        xt_h = xpool.tile([128, NT], F16, tag="xt")
        nc.sync.dma_start(xt_h[:, 0:HB], io["xt"][:, 0:HB])
        l_h, r_h = gates(xt_h, HB)
        # Core 0, chunk 0 (partitions 0-63, even cols): force l=1, r=0 over
        # the halo so the state is exactly 0 entering the first real step.
        # m/mm are per-partition (ones/zeros on other cores & partitions).
        lh3 = l_h[:, 2 : 2 + HB].rearrange("p (k s) -> p k s", s=2)
        rh3 = r_h[:, 2 : 2 + HB].rearrange("p (k s) -> p k s", s=2)
        nc.vector.tensor_scalar(
            lh3[:, :, 0:1], lh3[:, :, 0:1], m, mm, op0=OP.mult, op1=OP.add
        )
        nc.vector.tensor_scalar(
            rh3[:, :, 0:1], rh3[:, :, 0:1], m, None, op0=OP.mult
        )
        nc.vector.tensor_copy(l_h[:, 0:2], zeros2)  # seed carries = 0
        z_prev = zpool.tile([128, NT + 2], F16, tag="z")
        nc.vector._custom_dve(
            FUSED_SCAN,
            out=z_prev[:, 0 : HB + 2],
            in0=l_h[:, 0 : HB + 2],
            in1=r_h[:, 0 : HB + 2],
        )
        prev_n = HB  # carries live at z_prev[:, prev_n : prev_n+2]

        # ---------------- main loop ----------------
        # short first tiles cut pipeline-fill latency; split last tile
        # shortens the serial scan+DMA drain
        sizes = [256, 256, 512, 1024] + [NT] * 14 + [NT // 2, NT // 2]
        assert sum(sizes) == NREAL
        c0 = HB
        for n in sizes:
            xt_t = xpool.tile([128, NT], F16, tag="xt")
            nc.sync.dma_start(xt_t[:, 0:n], io["xt"][:, c0 : c0 + n])
            l_t, r_t = gates(xt_t, n)
            nc.vector.tensor_copy(
                l_t[:, 0:2], z_prev[:, prev_n : prev_n + 2]
            )  # carry
            z_t = zpool.tile([128, NT + 2], F16, tag="z")
            nc.vector._custom_dve(
                FUSED_SCAN,
                out=z_t[:, 0 : n + 2],
                in0=l_t[:, 0 : n + 2],
                in1=r_t[:, 0 : n + 2],
            )
            nc.sync.dma_start(io["out"][:, c0 - HB : c0 - HB + n], z_t[:, 2 : 2 + n])
            z_prev, prev_n = z_t, n
            c0 += n


def prep_inputs(x, W1, B1, W2, B2, n_cores: int):
    """Host-side prep: per-core packed, transposed, 2-way column-interleaved
    x + block-diagonal fp16 W1 halves + biases/masks."""
    x = np.asarray(x, np.float32)
    W1 = np.asarray(W1, np.float32)
    B1 = np.asarray(B1, np.float32)

    W1L, W1R = W1[:, :NCH], W1[:, NCH:]
    w1bdl = np.zeros((128, 128), np.float16)
    w1bdl[:64, :64] = W1L
    w1bdl[64:, 64:] = W1L
    w1bdr = np.zeros((128, 128), np.float16)
    w1bdr[:64, :64] = W1R
    w1bdr[64:, 64:] = W1R
    c16 = np.zeros((128, 258), np.float16)
    c16[:, 0:128] = w1bdl
    c16[:, 128:256] = w1bdr  # cols 256:258 stay zero (seed zeros)
    b1l = np.tile(B1[0, :NCH], 2).reshape(128, 1).astype(np.float32)
    b1r = np.tile(B1[0, NCH:], 2).reshape(128, 1).astype(np.float32)

    in_maps = []
    for c in range(n_cores):
        xt = np.empty((128, NCOL), np.float16)
        for pb in (0, 1):
            for s in (0, 1):
                g0 = c * C + (2 * pb + s) * NQ
                if g0 - W < 0:  # core 0, chunk 0: zero-pad the halo
                    xa = np.concatenate(
                        [np.zeros((W - g0, NCH), np.float32), x[0 : g0 + NQ]], 0
                    )
                else:
                    xa = x[g0 - W : g0 + NQ]
                xt[64 * pb : 64 * pb + 64, s::2] = xa.T
        if c == 0:
            m = np.concatenate(
                [np.zeros(64, np.float32), np.ones(64, np.float32)]
            ).reshape(128, 1)
        else:
            m = np.ones((128, 1), np.float32)
        c32 = np.concatenate([b1l, b1r, m, 1.0 - m], axis=1).astype(np.float32)
        in_maps.append(
            {
                "xt": np.ascontiguousarray(xt),
                "c16": c16,
                "c32": np.ascontiguousarray(c32),
            }
        )
    return in_maps


def declare_io(nc):
    io = {
        "xt": nc.dram_tensor("xt", [128, NCOL], F16, kind="ExternalInput"),
        "c16": nc.dram_tensor("c16", [128, 258], F16, kind="ExternalInput"),
        "c32": nc.dram_tensor("c32", [128, 4], F32, kind="ExternalInput"),
        "out": nc.dram_tensor("out", [128, NREAL], F16, kind="ExternalOutput"),
    }
    return {k: v.ap() for k, v in io.items()}


_NC = None
LAST_RESULTS = None


def _get_nc():
    global _NC
    if _NC is None:
        nc = bacc.Bacc(
            "TRN2", target_bir_lowering=False, debug=False, num_devices=N_CORES
        )
        io = declare_io(nc)
        with tile.TileContext(nc) as tc:
            build_deductron(tc, io)
        nc.compile()
        _NC = nc
    return _NC


def kernel(inputs, W1, B1, W2, B2):
    global LAST_RESULTS
    nc = _get_nc()
    in_maps = prep_inputs(inputs, W1, B1, W2, B2, N_CORES)
    trace = bool(int(os.environ.get("KERNEL_TRACE", "0")))
    res = run_bass_kernel_spmd(
        nc, in_maps, core_ids=list(range(N_CORES)), trace=trace
    )
    LAST_RESULTS = res
    # device emitted z (packed/transposed/interleaved fp16, shifted by one
    # row); finish z @ W2 + B2 here
    W2f = np.asarray(W2, np.float32)
    B2f = np.asarray(B2, np.float32).reshape(-1)
    z = np.empty((T + 1, 64), np.float32)
    z[0] = 0.0
    for c in range(N_CORES):
        zc = res.results[c]["out"]  # [128, NREAL] f16
        for pb in (0, 1):
            v = zc[64 * pb : 64 * pb + 64].reshape(64, NQ, 2)
            for s in (0, 1):
                g0 = c * C + (2 * pb + s) * NQ
                z[g0 + 1 : g0 + NQ + 1] = v[:, :, s].T
    return (z[:T] @ W2f + B2f).astype(np.float32)


# revision 5
# speedup vs baseline: 1.4792x; 1.0133x over previous
"""Deductron (sigmoid-gated affine linear recurrence) — Trainium2 Bass kernel.

Problem: T=524288, INPUT_LEN=64, N_MEMORY=64, OUTPUT_LEN=32.
  h = sigmoid(x @ W1 + B1); l, r = split(h); a = (l*r)[:-1]; b = (1-l)[:-1]
  u_t = a_{t-1} u_{t-1} + b_{t-1}, u_0 = 0;  out = z @ W2 + B2

Strategy (8 NeuronCores, sequence-parallel, no collectives):
  - a_t = sigmoid*sigmoid decays geometrically; a warm-up halo of W=128
    steps makes chunks independent to f32 precision. Core 0's chunk-0 halo
    is forced to (a=0, b=0) via l:=1, r:=0 with a per-core mask input.
  - Each core handles C=65536 rows as FOUR chunks of NQ=16384: 2 chunks in
    the partition dim (128 partitions = 2 blocks x 64 channels) and 2 chunks
    INTERLEAVED along columns (even cols = stream A, odd = stream B).
  - Gating: block-diagonal W1-half matmuls (fp16, K=128), two sigmoids on
    ScalarE (l and r) — the bottleneck engine at ~1 elem/cy/lane.
  - The recurrence runs in ONE custom DVE instruction per tile
    (DEDUCTRON_ISCAN2_ANT, see _register_fused_scan): a hand-written uOp
    program computing u <- (l*r)*u + (1-l) over the two interleaved streams
    at 1 elem/cycle — the stock tensor_tensor_scan needs a bubble uOp
    (~2.1 cy/elem) because its feedback distance is 1 element; interleaving
    two streams makes the NEXT_ALU_OUT_A feedback distance (2 elements)
    exactly right. This also fuses a=l*r and b=1-l, freeing DVE and ScalarE.
  - Seed convention: scan cols 0,1 = per-stream carries, passed via the l
    stream (copied from the previous tile's last two states).
  - Output: z streams to DRAM as fp16; the host finishes z @ W2 + B2 and
    the de-interleave during gather.
"""

import os
import sys

for _p in ("/opt/trn_rl_repo",):
    if _p not in sys.path and os.path.isdir(_p):
        sys.path.insert(0, _p)

import numpy as np

import concourse.bacc as bacc
import concourse.mybir as mybir
import concourse.tile as tile
from concourse.bass_utils import run_bass_kernel_spmd

F32 = mybir.dt.float32
F16 = mybir.dt.float16
AF = mybir.ActivationFunctionType
OP = mybir.AluOpType

# ---------------------------------------------------------------------------
# Custom DVE op: fused 2-interleaved affine scan (hand-written uOp program).
#
# Semantics over one [P, N] pair of streams l (Src0), r (Src1):
#     out[:, 0] = l[:, 0]; out[:, 1] = l[:, 1]          # seeds (carries)
#     out[:, k] = (l*r)[:, k] * out[:, k-2] + (1-l)[:, k]   for k >= 2
# fp32 internal state, 1 element/cycle.
#
# Datapath (v3): state = blk3's a_flop, read by blk2 via NEXT_ALU_OUT_A —
# with elements issued every cycle, the value read is 2 elements stale,
# which is exactly the same-stream predecessor under 2-way interleave.
#     lanes: inp0=SRC_0(l), inp1=SRC_1(r)→ch0, inp2=ONE→ch1, inp3=ZERO→ch2
#     blk0: p = l * r                       capture l→ch3
#     blk1: t = 1 - l    (SUB ch1, ch3)     capture p→ch4
#     blk2: m = p * state (MUL ch4, NEXT_A) capture t→ch5
#     blk3: u = m + t  → out flop + a_flop
#     blk4-7: bypass → WR0_LO
# Seed uOp (repeat 2, consumes + writes): blk1 = BYPASS(ch3)=l,
# blk2 = BYPASS(ch2)=0 — never reads the stale a_flop.
# ---------------------------------------------------------------------------


def _register_fused_scan():
    from concourse.dve_ops import (
        OPS,
        CUSTOM_DVE_SPECS,
        _SUB_OPCODE_FOR_NAME,
        DveOp,
        get_dve_sub_opcode,
    )
    from concourse.dve_spec import Spec, Src0, Src1
    from concourse.dve_uop import (
        ENABLE,
        AluInp,
        AluOp,
        DelayInp,
        DveOpSpec,
        InpSel,
        OutPath,
        OutSel,
        Trigger,
        UopConfig,
        UopDpConfig,
    )

    name = "DEDUCTRON_ISCAN2_ANT"
    if name in _SUB_OPCODE_FOR_NAME:  # already registered (re-import)
        return next(op for op in OPS if op.name == name)

    def _dp():
        dp = [UopDpConfig() for _ in range(8)]
        dp[0].enable_alu(AluOp.MULTIPLY, AluInp.PREV_ALU_OUT, AluInp.PREV_DELAY_0)
        dp[0].pass_through_delay(1, 2)
        dp[0].enable_delay_from_src(DelayInp.PREV_ALU_OUT, 3)
        dp[1].enable_alu(AluOp.SUBTRACT, AluInp.PREV_DELAY_1, AluInp.PREV_DELAY_3)
        dp[1].pass_through_delay(2)
        dp[1].enable_delay_from_src(DelayInp.PREV_ALU_OUT, 4)
        dp[2].enable_alu(AluOp.MULTIPLY, AluInp.PREV_DELAY_4, AluInp.NEXT_ALU_OUT_A)
        dp[2].enable_delay_from_src(DelayInp.PREV_ALU_OUT, 5)
        dp[3].enable_alu(AluOp.ADD, AluInp.PREV_ALU_OUT, AluInp.PREV_DELAY_5)
        dp[3].alu_out_a_enable = ENABLE
        for i in range(4, 8):
            dp[i].pass_through_alu()
        return dp

    def _uop(dp):
        u = UopConfig(datapath_config=dp)
        u.enable_input(InpSel.SRC_0, 0)
        u.enable_input(InpSel.SRC_1, 1)
        u.enable_input(InpSel.ONE_F32, 2)
        u.enable_input(InpSel.ZERO, 3)
        u.enable_output(OutSel.ALU_OUT, OutPath.WR0_LO)
        u.require_inp0 = 1
        u.require_inp1 = 1
        return u

    def _uops():
        seed = _uop(_dp())
        seed.datapath_config[1].enable_alu(
            AluOp.BYPASS, AluInp.PREV_DELAY_3, AluInp.PREV_DELAY_3
        )
        seed.datapath_config[2].enable_alu(
            AluOp.BYPASS, AluInp.PREV_DELAY_2, AluInp.PREV_DELAY_2
        )
        seed.repeat_count = 2
        seed.trigger = (Trigger.COUNT, Trigger.NONE, Trigger.NONE)
        seed.next_uop = (1, 0, 0)
        steady = _uop(_dp())
        steady.trigger = (Trigger.SRC_TENSOR_DONE, Trigger.NONE, Trigger.NONE)
        steady.next_uop = (0, 0, 0)
        return [seed, steady]

    def _reference(in0, in1, c0, c1, c2):
        l = np.asarray(in0, np.float32)
        r = np.asarray(in1, np.float32)
        P = l.shape[0]
        l2, r2 = l.reshape(P, -1), r.reshape(P, -1)
        out = np.empty_like(l2)
        out[:, 0:2] = l2[:, 0:2]
        for k in range(2, l2.shape[1]):
            out[:, k] = l2[:, k] * r2[:, k] * out[:, k - 2] + (1.0 - l2[:, k])
        return out.reshape(l.shape)

    class _HandDveOp(DveOp):
        def compile(self, ver):
            assert ver == "v3", f"{self.name}: hand-written program targets v3/TRN2"
            s = DveOpSpec(
                name=self.name,
                opcode=get_dve_sub_opcode(self.name),
                uops=_uops(),
                rd1_en=True,
            )
            s.validate(ver)
            return s

    op = _HandDveOp(
        name, Spec(body=Src0 * Src1, reference=_reference), subdim=False, uops_sha={}
    )
    row = max(_SUB_OPCODE_FOR_NAME.values()) + 1
    assert row < 0x20
    _SUB_OPCODE_FOR_NAME[name] = row
    OPS.append(op)
    CUSTOM_DVE_SPECS[name] = op.spec
    return op


FUSED_SCAN = _register_fused_scan()

# ---------------------------------------------------------------------------
# Kernel
# ---------------------------------------------------------------------------

N_CORES = 8
T = 524288
NCH = 64
C = T // N_CORES  # 65536 rows per core
NQ = C // 4  # 16384 rows per chunk (4 chunks per core)
W = 128  # warm-up halo steps per chunk
HB = 2 * W  # interleaved halo columns
NCOL = 2 * (W + NQ)  # xt columns per core (33024)
NREAL = 2 * NQ  # real (output) columns per core (32768)
NT = 2048  # main-loop tile columns


def build_deductron(tc, io):
    """io: dict of DRAM APs: xt [128, NCOL] f16, c16 [128, 258] f16
    (w1bdl | w1bdr | zeros2), c32 [128, 4] f32 (b1l | b1r | m | mm),
    out [128, NREAL] f16 (interleaved z, shifted by one row)."""
    nc = tc.nc

    with (
        tc.tile_pool(name="consts", bufs=1) as cpool,
        tc.tile_pool(name="xt", bufs=4) as xpool,
        tc.tile_pool(name="lr", bufs=3) as lrpool,
        tc.tile_pool(name="z", bufs=3) as zpool,
        tc.tile_pool(name="pzl", bufs=1, space="PSUM") as pzl,
        tc.tile_pool(name="pzr", bufs=1, space="PSUM") as pzr,
    ):
        c16 = cpool.tile([128, 258], F16, tag="c16")
        c32 = cpool.tile([128, 4], F32, tag="c32")
        nc.sync.dma_start(c16[:], io["c16"])
        nc.sync.dma_start(c32[:], io["c32"])
        w1bdl, w1bdr, zeros2 = c16[:, 0:128], c16[:, 128:256], c16[:, 256:258]
        b1l, b1r = c32[:, 0:1], c32[:, 1:2]
        m, mm = c32[:, 2:3], c32[:, 3:4]

        def gates(xt_t, n):
            """matmul + sigmoid for both halves; returns l_t, r_t with the
            gate values at cols [2, 2+n) (cols 0,1 reserved for carries).
            Matmuls are grouped by weight so the PE array reloads weights
            once per half, not once per 512-chunk."""
            l_t = lrpool.tile([128, NT + 2], F16, tag="l")
            r_t = lrpool.tile([128, NT + 2], F16, tag="r")
            zl_t = pzl.tile([128, NT], F32, tag="zl")
            zr_t = pzr.tile([128, NT], F32, tag="zr")
            for q0 in range(0, n, 512):
                q1 = min(q0 + 512, n)
                nc.tensor.matmul(
                    zl_t[:, q0:q1], w1bdl, xt_t[:, q0:q1], start=True, stop=True
                )
            nc.scalar.activation(
                l_t[:, 2 : 2 + n], zl_t[:, 0:n], AF.Sigmoid, bias=b1l
            )
            for q0 in range(0, n, 512):
                q1 = min(q0 + 512, n)
                nc.tensor.matmul(
                    zr_t[:, q0:q1], w1bdr, xt_t[:, q0:q1], start=True, stop=True
                )
            nc.scalar.activation(
                r_t[:, 2 : 2 + n], zr_t[:, 0:n], AF.Sigmoid, bias=b1r
            )
            return l_t, r_t

        # ---------------- main loop ----------------
        # tile 0 contains the HB halo columns plus its first real columns;
        # short first tiles cut pipeline-fill latency; split last tile
        # shortens the serial scan+DMA drain
        sizes = [HB + 512, 512, 1024] + [NT] * 14 + [NT // 2, NT // 2]
        assert sum(sizes) == NCOL
        z_prev, prev_n = None, 0
        c0 = 0
        for it, n in enumerate(sizes):
            xt_t = xpool.tile([128, NT], F16, tag="xt")
            nc.sync.dma_start(xt_t[:, 0:n], io["xt"][:, c0 : c0 + n])
            l_t, r_t = gates(xt_t, n)
            if it == 0:
                # Core 0, chunk 0 (partitions 0-63, even cols): force l=1,
                # r=0 over the halo so the state is exactly 0 entering the
                # first real step. m/mm are per-partition (ones/zeros on
                # other cores & partitions).
                lh3 = l_t[:, 2 : 2 + HB].rearrange("p (k s) -> p k s", s=2)
                rh3 = r_t[:, 2 : 2 + HB].rearrange("p (k s) -> p k s", s=2)
                nc.vector.tensor_scalar(
                    lh3[:, :, 0:1], lh3[:, :, 0:1], m, mm, op0=OP.mult, op1=OP.add
                )
                nc.vector.tensor_scalar(
                    rh3[:, :, 0:1], rh3[:, :, 0:1], m, None, op0=OP.mult
                )
                nc.vector.tensor_copy(l_t[:, 0:2], zeros2)  # seed carries = 0
            else:
                nc.vector.tensor_copy(
                    l_t[:, 0:2], z_prev[:, prev_n : prev_n + 2]
                )  # carry
            z_t = zpool.tile([128, NT + 2], F16, tag="z")
            nc.vector._custom_dve(
                FUSED_SCAN,
                out=z_t[:, 0 : n + 2],
                in0=l_t[:, 0 : n + 2],
                in1=r_t[:, 0 : n + 2],
            )
            # skip the halo columns on the way out (tile 0 only)
            skip = HB if it == 0 else 0
            nc.sync.dma_start(
                io["out"][:, c0 - HB + skip : c0 - HB + n],
                z_t[:, 2 + skip : 2 + n],
            )
            z_prev, prev_n = z_t, n
            c0 += n


def prep_inputs(x, W1, B1, W2, B2, n_cores: int):
    """Host-side prep: per-core packed, transposed, 2-way column-interleaved
    x + block-diagonal fp16 W1 halves + biases/masks."""
    x = np.asarray(x, np.float32)
    W1 = np.asarray(W1, np.float32)
    B1 = np.asarray(B1, np.float32)

    W1L, W1R = W1[:, :NCH], W1[:, NCH:]
    w1bdl = np.zeros((128, 128), np.float16)
    w1bdl[:64, :64] = W1L
    w1bdl[64:, 64:] = W1L
    w1bdr = np.zeros((128, 128), np.float16)
    w1bdr[:64, :64] = W1R
    w1bdr[64:, 64:] = W1R
    c16 = np.zeros((128, 258), np.float16)
    c16[:, 0:128] = w1bdl
    c16[:, 128:256] = w1bdr  # cols 256:258 stay zero (seed zeros)
    b1l = np.tile(B1[0, :NCH], 2).reshape(128, 1).astype(np.float32)
    b1r = np.tile(B1[0, NCH:], 2).reshape(128, 1).astype(np.float32)

    in_maps = []
    for c in range(n_cores):
        xt = np.empty((128, NCOL), np.float16)
        for pb in (0, 1):
            for s in (0, 1):
                g0 = c * C + (2 * pb + s) * NQ
                if g0 - W < 0:  # core 0, chunk 0: zero-pad the halo
                    xa = np.concatenate(
                        [np.zeros((W - g0, NCH), np.float32), x[0 : g0 + NQ]], 0
                    )
                else:
                    xa = x[g0 - W : g0 + NQ]
                xt[64 * pb : 64 * pb + 64, s::2] = xa.T
        if c == 0:
            m = np.concatenate(
                [np.zeros(64, np.float32), np.ones(64, np.float32)]
            ).reshape(128, 1)
        else:
            m = np.ones((128, 1), np.float32)
        c32 = np.concatenate([b1l, b1r, m, 1.0 - m], axis=1).astype(np.float32)
        in_maps.append(
            {
                "xt": np.ascontiguousarray(xt),
                "c16": c16,
                "c32": np.ascontiguousarray(c32),
            }
        )
    return in_maps


def declare_io(nc):
    io = {
        "xt": nc.dram_tensor("xt", [128, NCOL], F16, kind="ExternalInput"),
        "c16": nc.dram_tensor("c16", [128, 258], F16, kind="ExternalInput"),
        "c32": nc.dram_tensor("c32", [128, 4], F32, kind="ExternalInput"),
        "out": nc.dram_tensor("out", [128, NREAL], F16, kind="ExternalOutput"),
    }
    return {k: v.ap() for k, v in io.items()}


_NC = None
LAST_RESULTS = None


def _get_nc():
    global _NC
    if _NC is None:
        nc = bacc.Bacc(
            "TRN2", target_bir_lowering=False, debug=False, num_devices=N_CORES
        )
        io = declare_io(nc)
        with tile.TileContext(nc) as tc:
            build_deductron(tc, io)
        nc.compile()
        _NC = nc
    return _NC


def kernel(inputs, W1, B1, W2, B2):
    global LAST_RESULTS
    nc = _get_nc()
    in_maps = prep_inputs(inputs, W1, B1, W2, B2, N_CORES)
    trace = bool(int(os.environ.get("KERNEL_TRACE", "0")))
    res = run_bass_kernel_spmd(
        nc, in_maps, core_ids=list(range(N_CORES)), trace=trace
    )
    LAST_RESULTS = res
    # device emitted z (packed/transposed/interleaved fp16, shifted by one
    # row); finish z @ W2 + B2 here
    W2f = np.asarray(W2, np.float32)
    B2f = np.asarray(B2, np.float32).reshape(-1)
    z = np.empty((T + 1, 64), np.float32)
    z[0] = 0.0
    for c in range(N_CORES):
        zc = res.results[c]["out"]  # [128, NREAL] f16
        for pb in (0, 1):
            v = zc[64 * pb : 64 * pb + 64].reshape(64, NQ, 2)
            for s in (0, 1):
                g0 = c * C + (2 * pb + s) * NQ
                z[g0 + 1 : g0 + NQ + 1] = v[:, :, s].T
    return (z[:T] @ W2f + B2f).astype(np.float32)


# revision 6
# speedup vs baseline: 1.5055x; 1.0178x over previous
"""Deductron (sigmoid-gated affine linear recurrence) — Trainium2 Bass kernel.

Problem: T=524288, INPUT_LEN=64, N_MEMORY=64, OUTPUT_LEN=32.
  h = sigmoid(x @ W1 + B1); l, r = split(h); a = (l*r)[:-1]; b = (1-l)[:-1]
  u_t = a_{t-1} u_{t-1} + b_{t-1}, u_0 = 0;  out = z @ W2 + B2

Strategy (8 NeuronCores, sequence-parallel, no collectives):
  - a_t = sigmoid*sigmoid decays geometrically; a warm-up halo of W=128
    steps makes chunks independent to f32 precision. Core 0's chunk-0 halo
    is forced to (a=0, b=0) via l:=1, r:=0 with a per-core mask input.
  - Each core handles C=65536 rows as FOUR chunks of NQ=16384: 2 chunks in
    the partition dim (128 partitions = 2 blocks x 64 channels) and 2 chunks
    INTERLEAVED along columns (even cols = stream A, odd = stream B).
  - Gating: block-diagonal W1-half matmuls (fp16, K=128). The l gate is a
    ScalarE sigmoid; the r gate is kept in tanh form (sigma(x) = 0.5 +
    0.5*tanh(x/2)): most columns via a ScalarE Tanh, the tail fraction of
    each tile via a custom DVE degree-5 odd polynomial straight from PSUM —
    ScalarE (1 elem/cy/lane, the bottleneck) and DVE finish together.
  - The recurrence runs in ONE custom DVE instruction per tile
    (DEDUCTRON_ISCAN2T_ANT): a hand-written uOp program computing
    u <- l*(0.5*rt+0.5)*u + (1-l) over the two interleaved streams at
    1 elem/cycle — the stock tensor_tensor_scan needs a bubble uOp
    (~2.1 cy/elem) because its feedback distance is 1 element; interleaving
    two streams makes the NEXT_ALU_OUT_A feedback distance (2 elements)
    exactly right. This also fuses a=l*r and b=1-l, freeing DVE and ScalarE.
  - Seed convention: scan cols 0,1 = per-stream carries, passed via the l
    stream (copied from the previous tile's last two states).
  - Output: z streams to DRAM as fp16; the host finishes z @ W2 + B2 and
    the de-interleave during gather.
"""

import os
import sys

for _p in ("/opt/trn_rl_repo",):
    if _p not in sys.path and os.path.isdir(_p):
        sys.path.insert(0, _p)

import numpy as np

import concourse.bacc as bacc
import concourse.mybir as mybir
import concourse.tile as tile
from concourse.bass_utils import run_bass_kernel_spmd

F32 = mybir.dt.float32
F16 = mybir.dt.float16
AF = mybir.ActivationFunctionType
OP = mybir.AluOpType

# tanh(u) ~ u*(a1 + a3 u^2 + a5 u^4) minimax-ish on [-1.8, 1.8];
# reparam v = q*zr + (q*b1r), q = 0.5*a1: y = v + c3 v^3 + c5 v^5.
POLY_Q = 0.4843168686709407
POLY_C3 = -0.25200501704078765
POLY_C5 = 0.03388509147965195


def _register_ops():
    """Register the two custom DVE ops (per-NEFF uOp table).

    Op DEDUCTRON_ISCAN2T_ANT (hand-written uOp program):
        out[:,0] = l[:,0]; out[:,1] = l[:,1]              # seeds (carries)
        out[:,k] = l[:,k]*(0.5*rt[:,k]+0.5)*out[:,k-2] + (1-l[:,k])
      at 1 elem/cycle, fp32 state in blk5's a_flop read by blk4 via
      NEXT_ALU_OUT_A (2-element feedback = the 2-way column interleave).
      s0 (CONST_0) must be 0.5 at the call site.

    Op DEDUCTRON_TANHP_ANT (Spec-compiled):
        v = zr*C0 + C3(in1, per-partition); out = v + (v*v^2)*(C1 + v^2*C2)
    """
    from concourse.dve_ops import (
        OPS,
        CUSTOM_DVE_SPECS,
        _SUB_OPCODE_FOR_NAME,
        DveOp,
        get_dve_sub_opcode,
    )
    from concourse.dve_spec import (
        C0,
        C1,
        C2,
        C3,
        Spec,
        Src0,
        Src1,
        _spill_c3_to_src1,
        lower,
        sq,
    )
    from concourse.dve_uop import (
        ENABLE,
        AluInp,
        AluOp,
        DelayInp,
        DveOpSpec,
        InpSel,
        OutPath,
        OutSel,
        Trigger,
        UopConfig,
        UopDpConfig,
    )

    def _scan_dp():
        # lanes: 0=SRC_0(l)->ALU, ch0=SRC_1(rt), ch1=ONE, ch2=ZERO, ch3=CONST_0
        # blk0: h = rt*0.5 (capture l->ch4); blk1: s = h+0.5; blk2: p = s*l;
        # blk3: t = 1-l (capture p->ch5); blk4: m = p*state (capture t->ch4);
        # blk5: u = m+t -> out + a_flop; blk6-7 bypass.
        dp = [UopDpConfig() for _ in range(8)]
        dp[0].enable_alu(AluOp.MULTIPLY, AluInp.PREV_DELAY_0, AluInp.PREV_DELAY_3)
        dp[0].pass_through_delay(1, 2, 3)
        dp[0].enable_delay_from_src(DelayInp.PREV_ALU_OUT, 4)
        dp[1].enable_alu(AluOp.ADD, AluInp.PREV_ALU_OUT, AluInp.PREV_DELAY_3)
        dp[1].pass_through_delay(1, 2, 4)
        dp[2].enable_alu(AluOp.MULTIPLY, AluInp.PREV_ALU_OUT, AluInp.PREV_DELAY_4)
        dp[2].pass_through_delay(1, 2, 4)
        dp[3].enable_alu(AluOp.SUBTRACT, AluInp.PREV_DELAY_1, AluInp.PREV_DELAY_4)
        dp[3].pass_through_delay(2)
        dp[3].enable_delay_from_src(DelayInp.PREV_ALU_OUT, 5)
        dp[4].enable_alu(AluOp.MULTIPLY, AluInp.PREV_DELAY_5, AluInp.NEXT_ALU_OUT_A)
        dp[4].enable_delay_from_src(DelayInp.PREV_ALU_OUT, 4)
        dp[5].enable_alu(AluOp.ADD, AluInp.PREV_ALU_OUT, AluInp.PREV_DELAY_4)
        dp[5].alu_out_a_enable = ENABLE
        for i in range(6, 8):
            dp[i].pass_through_alu()
        return dp

    def _scan_uop(dp):
        u = UopConfig(datapath_config=dp)
        u.enable_input(InpSel.SRC_0, 0)
        u.enable_input(InpSel.SRC_1, 1)
        u.enable_input(InpSel.ONE_F32, 2)
        u.enable_input(InpSel.ZERO, 3)
        u.enable_input(InpSel.CONST_0, 4)
        u.enable_output(OutSel.ALU_OUT, OutPath.WR0_LO)
        u.require_inp0 = 1
        u.require_inp1 = 1
        return u

    def _scan_uops():
        seed = _scan_uop(_scan_dp())
        seed.datapath_config[3].enable_alu(
            AluOp.BYPASS, AluInp.PREV_DELAY_4, AluInp.PREV_DELAY_4
        )
        seed.datapath_config[3].enable_delay_from_src(DelayInp.PREV_ALU_OUT, 5)
        seed.datapath_config[4].enable_alu(
            AluOp.BYPASS, AluInp.PREV_DELAY_2, AluInp.PREV_DELAY_2
        )
        seed.repeat_count = 2
        seed.trigger = (Trigger.COUNT, Trigger.NONE, Trigger.NONE)
        seed.next_uop = (1, 0, 0)
        steady = _scan_uop(_scan_dp())
        steady.trigger = (Trigger.SRC_TENSOR_DONE, Trigger.NONE, Trigger.NONE)
        steady.next_uop = (0, 0, 0)
        return [seed, steady]

    def _scan_reference(in0, in1, c0, c1, c2):
        l = np.asarray(in0, np.float32)
        rt = np.asarray(in1, np.float32)
        P = l.shape[0]
        l2, r2 = l.reshape(P, -1), rt.reshape(P, -1)
        out = np.empty_like(l2)
        out[:, 0:2] = l2[:, 0:2]
        for k in range(2, l2.shape[1]):
            sig = 0.5 * r2[:, k] + 0.5
            out[:, k] = l2[:, k] * sig * out[:, k - 2] + (1.0 - l2[:, k])
        return out.reshape(l.shape)

    class _HandDveOp(DveOp):
        def compile(self, ver):
            assert ver == "v3", f"{self.name}: hand-written program targets v3/TRN2"
            s = DveOpSpec(
                name=self.name,
                opcode=get_dve_sub_opcode(self.name),
                uops=_scan_uops(),
                rd1_en=True,
            )
            s.validate(ver)
            return s

    class _SpecDveOp(DveOp):
        def compile(self, ver):
            return DveOpSpec(
                name=self.name,
                opcode=get_dve_sub_opcode(self.name),
                uops=lower(self.spec, ver=ver),
                rd1_en=True,
            )

    def _poly_reference(in0, in1, c0, c1, c2):
        v = np.asarray(in0, np.float32) * c0 + np.asarray(in1, np.float32).reshape(
            -1, 1
        )
        v2 = v * v
        return v + (v * v2) * (c1 + v2 * c2)

    _v = Src0 * C0 + C3
    _v2 = sq(_v)
    new_ops = [
        _HandDveOp(
            "DEDUCTRON_ISCAN2T_ANT",
            Spec(body=Src0 * Src1, reference=_scan_reference),
            subdim=False,
            uops_sha={},
        ),
        _SpecDveOp(
            "DEDUCTRON_TANHP_ANT",
            Spec(
                body=_spill_c3_to_src1(_v + (_v * _v2) * (C1 + _v2 * C2)),
                reference=_poly_reference,
            ),
            subdim=False,
            uops_sha={},
        ),
    ]
    out = []
    for op in new_ops:
        if op.name not in _SUB_OPCODE_FOR_NAME:
            row = max(_SUB_OPCODE_FOR_NAME.values()) + 1
            assert row < 0x20
            _SUB_OPCODE_FOR_NAME[op.name] = row
            OPS.append(op)
            CUSTOM_DVE_SPECS[op.name] = op.spec
        else:
            op = next(o for o in OPS if o.name == op.name)
        out.append(op)
    return out


FUSED_SCAN, TANH_POLY = _register_ops()

# ---------------------------------------------------------------------------
# Kernel
# ---------------------------------------------------------------------------

N_CORES = 8
T = 524288
NCH = 64
C = T // N_CORES  # 65536 rows per core
NQ = C // 4  # 16384 rows per chunk (4 chunks per core)
W = 128  # warm-up halo steps per chunk
HB = 2 * W  # interleaved halo columns
NCOL = 2 * (W + NQ)  # xt columns per core (33024)
NREAL = 2 * NQ  # real (output) columns per core (32768)
NT = 2048  # main-loop tile columns


def _poly_cols(n):
    """Tail columns of each tile whose r-gate runs on DVE instead of ScalarE
    (load balance: ScalarE 2 sigmoids/tile is otherwise the bottleneck)."""
    return (n // 4) & ~1 if n >= 1024 else 0


def build_deductron(tc, io):
    """io: dict of DRAM APs: xt [128, NCOL] f16, c16 [128, 258] f16
    (w1bdl | w1bdr | zeros2), c32 [128, 6] f32 (b1l | b1rh | m | mm | mneg |
    qb1r), out [128, NREAL] f16 (interleaved z, shifted by one row)."""
    nc = tc.nc

    with (
        tc.tile_pool(name="consts", bufs=1) as cpool,
        tc.tile_pool(name="xt", bufs=4) as xpool,
        tc.tile_pool(name="lr", bufs=3) as lrpool,
        tc.tile_pool(name="z", bufs=3) as zpool,
        tc.tile_pool(name="pzl", bufs=1, space="PSUM") as pzl,
        tc.tile_pool(name="pzr", bufs=1, space="PSUM") as pzr,
    ):
        c16 = cpool.tile([128, 258], F16, tag="c16")
        c32 = cpool.tile([128, 6], F32, tag="c32")
        nc.sync.dma_start(c16[:], io["c16"])
        nc.sync.dma_start(c32[:], io["c32"])
        w1bdl, w1bdr, zeros2 = c16[:, 0:128], c16[:, 128:256], c16[:, 256:258]
        b1l, b1rh = c32[:, 0:1], c32[:, 1:2]
        m, mm, mneg = c32[:, 2:3], c32[:, 3:4], c32[:, 4:5]
        qb1r = c32[:, 5:6]

        def gates(xt_t, n):
            """matmuls + l sigmoid + r tanh (ScalarE head, DVE-poly tail);
            gate values land at cols [2, 2+n) (cols 0,1 = carries)."""
            pc = _poly_cols(n)
            l_t = lrpool.tile([128, NT + 2], F16, tag="l")
            r_t = lrpool.tile([128, NT + 2], F16, tag="r")
            zl_t = pzl.tile([128, NT], F32, tag="zl")
            zr_t = pzr.tile([128, NT], F32, tag="zr")
            for q0 in range(0, n, 512):
                q1 = min(q0 + 512, n)
                nc.tensor.matmul(
                    zl_t[:, q0:q1], w1bdl, xt_t[:, q0:q1], start=True, stop=True
                )
            nc.scalar.activation(
                l_t[:, 2 : 2 + n], zl_t[:, 0:n], AF.Sigmoid, bias=b1l
            )
            for q0 in range(0, n, 512):
                q1 = min(q0 + 512, n)
                nc.tensor.matmul(
                    zr_t[:, q0:q1], w1bdr, xt_t[:, q0:q1], start=True, stop=True
                )
            # r in tanh form: sigma(zr + b1r) = 0.5 + 0.5*tanh(0.5*zr + 0.5*b1r)
            nc.scalar.activation(
                r_t[:, 2 : 2 + n - pc],
                zr_t[:, 0 : n - pc],
                AF.Tanh,
                bias=b1rh,
                scale=0.5,
            )
            if pc:
                nc.vector._custom_dve(
                    TANH_POLY,
                    out=r_t[:, 2 + n - pc : 2 + n],
                    in0=zr_t[:, n - pc : n],
                    in1=qb1r,
                    s0=POLY_Q,
                    s1=POLY_C3,
                    imm2=POLY_C5,
                )
            return l_t, r_t

        # ---------------- main loop ----------------
        # tile 0 contains the HB halo columns plus its first real columns;
        # short first tiles cut pipeline-fill latency; split last tile
        # shortens the serial scan+DMA drain
        sizes = [HB + 512, 512, 1024] + [NT] * 14 + [NT // 2, NT // 2]
        assert sum(sizes) == NCOL
        z_prev, prev_n = None, 0
        c0 = 0
        for it, n in enumerate(sizes):
            xt_t = xpool.tile([128, NT], F16, tag="xt")
            nc.sync.dma_start(xt_t[:, 0:n], io["xt"][:, c0 : c0 + n])
            l_t, r_t = gates(xt_t, n)
            if it == 0:
                # Core 0, chunk 0 (partitions 0-63, even cols): force l=1,
                # rt=-1 (sigma_r=0) over the halo so the state is exactly 0
                # entering the first real step. m/mm/mneg are per-partition
                # (no-ops on other cores & partitions).
                lh3 = l_t[:, 2 : 2 + HB].rearrange("p (k s) -> p k s", s=2)
                rh3 = r_t[:, 2 : 2 + HB].rearrange("p (k s) -> p k s", s=2)
                nc.vector.tensor_scalar(
                    lh3[:, :, 0:1], lh3[:, :, 0:1], m, mm, op0=OP.mult, op1=OP.add
                )
                nc.vector.tensor_scalar(
                    rh3[:, :, 0:1], rh3[:, :, 0:1], m, mneg, op0=OP.mult, op1=OP.add
                )
                nc.vector.tensor_copy(l_t[:, 0:2], zeros2)  # seed carries = 0
            else:
                nc.vector.tensor_copy(
                    l_t[:, 0:2], z_prev[:, prev_n : prev_n + 2]
                )  # carry
            z_t = zpool.tile([128, NT + 2], F16, tag="z")
            nc.vector._custom_dve(
                FUSED_SCAN,
                out=z_t[:, 0 : n + 2],
                in0=l_t[:, 0 : n + 2],
                in1=r_t[:, 0 : n + 2],
                s0=0.5,
            )
            # skip the halo columns on the way out (tile 0 only)
            skip = HB if it == 0 else 0
            nc.sync.dma_start(
                io["out"][:, c0 - HB + skip : c0 - HB + n],
                z_t[:, 2 + skip : 2 + n],
            )
            z_prev, prev_n = z_t, n
            c0 += n


def prep_inputs(x, W1, B1, W2, B2, n_cores: int):
    """Host-side prep: per-core packed, transposed, 2-way column-interleaved
    x + block-diagonal fp16 W1 halves + biases/masks."""
    x = np.asarray(x, np.float32)
    W1 = np.asarray(W1, np.float32)
    B1 = np.asarray(B1, np.float32)

    W1L, W1R = W1[:, :NCH], W1[:, NCH:]
    w1bdl = np.zeros((128, 128), np.float16)
    w1bdl[:64, :64] = W1L
    w1bdl[64:, 64:] = W1L
    w1bdr = np.zeros((128, 128), np.float16)
    w1bdr[:64, :64] = W1R
    w1bdr[64:, 64:] = W1R
    c16 = np.zeros((128, 258), np.float16)
    c16[:, 0:128] = w1bdl
    c16[:, 128:256] = w1bdr  # cols 256:258 stay zero (seed zeros)
    b1l = np.tile(B1[0, :NCH], 2).reshape(128, 1).astype(np.float32)
    b1r = np.tile(B1[0, NCH:], 2).reshape(128, 1).astype(np.float32)

    in_maps = []
    for c in range(n_cores):
        xt = np.empty((128, NCOL), np.float16)
        for pb in (0, 1):
            for s in (0, 1):
                g0 = c * C + (2 * pb + s) * NQ
                if g0 - W < 0:  # core 0, chunk 0: zero-pad the halo
                    xa = np.concatenate(
                        [np.zeros((W - g0, NCH), np.float32), x[0 : g0 + NQ]], 0
                    )
                else:
                    xa = x[g0 - W : g0 + NQ]
                xt[64 * pb : 64 * pb + 64, s::2] = xa.T
        if c == 0:
            m = np.concatenate(
                [np.zeros(64, np.float32), np.ones(64, np.float32)]
            ).reshape(128, 1)
        else:
            m = np.ones((128, 1), np.float32)
        c32 = np.concatenate(
            [b1l, 0.5 * b1r, m, 1.0 - m, m - 1.0, POLY_Q * b1r], axis=1
        ).astype(np.float32)
        in_maps.append(
            {
                "xt": np.ascontiguousarray(xt),
                "c16": c16,
                "c32": np.ascontiguousarray(c32),
            }
        )
    return in_maps


def declare_io(nc):
    io = {
        "xt": nc.dram_tensor("xt", [128, NCOL], F16, kind="ExternalInput"),
        "c16": nc.dram_tensor("c16", [128, 258], F16, kind="ExternalInput"),
        "c32": nc.dram_tensor("c32", [128, 6], F32, kind="ExternalInput"),
        "out": nc.dram_tensor("out", [128, NREAL], F16, kind="ExternalOutput"),
    }
    return {k: v.ap() for k, v in io.items()}


_NC = None
LAST_RESULTS = None


def _get_nc():
    global _NC
    if _NC is None:
        nc = bacc.Bacc(
            "TRN2", target_bir_lowering=False, debug=False, num_devices=N_CORES
        )
        io = declare_io(nc)
        with tile.TileContext(nc) as tc:
            build_deductron(tc, io)
        nc.compile()
        _NC = nc
    return _NC


def kernel(inputs, W1, B1, W2, B2):
    global LAST_RESULTS
    nc = _get_nc()
    in_maps = prep_inputs(inputs, W1, B1, W2, B2, N_CORES)
    trace = bool(int(os.environ.get("KERNEL_TRACE", "0")))
    res = run_bass_kernel_spmd(
        nc, in_maps, core_ids=list(range(N_CORES)), trace=trace
    )
    LAST_RESULTS = res
    # device emitted z (packed/transposed/interleaved fp16, shifted by one
    # row); finish z @ W2 + B2 here
    W2f = np.asarray(W2, np.float32)
    B2f = np.asarray(B2, np.float32).reshape(-1)
    z = np.empty((T + 1, 64), np.float32)
    z[0] = 0.0
    for c in range(N_CORES):
        zc = res.results[c]["out"]  # [128, NREAL] f16
        for pb in (0, 1):
            v = zc[64 * pb : 64 * pb + 64].reshape(64, NQ, 2)
            for s in (0, 1):
                g0 = c * C + (2 * pb + s) * NQ
                z[g0 + 1 : g0 + NQ + 1] = v[:, :, s].T
    return (z[:T] @ W2f + B2f).astype(np.float32)
